# revision 48
# baseline (speedup 1.0000x reference)
"""DCRNN kernel for 8 Trainium2 NeuronCores (Bass/Tile).

Graph/data-parallel sharding: nodes permuted so core c owns batch-lanes
[c*125,(c+1)*125) of every graph; edges partitioned by dst shard and bucketed
by (dst-group of 128, src-block) with cross-core-uniform chunk counts so one
SPMD program serves all 8 cores. Aggregation = dma_gather (one big gather per
(supergroup, src-block) run, int16 per-block indices) + one-hot matmul scatter
accumulating in PSUM. The one-hot is built in an interleaved [p, dst, chunk]
layout so every DVE operand is packed 2-byte SBUF (4x DVE mode). conv1's
4-wide transposed agg is AllGathered (tiny); every core then recomputes full
h1 (relu split across Act+DVE) and writes a partition-major bf16 h1 table
(12.5KB DMA descriptors) for conv2's gather. conv2 aggregates transposed
(lhsT=gathered rows) so the deg scaling is a free-dim multiply and h2 comes
out column-major for the LSTM. LSTM (bf16 weights/states) is interleaved with
conv2 group-by-group so it hides under conv2's DMA; global mean pool via
free-dim reduce, partial logits AllReduce (800B) + on-core log_softmax.
"""
import os
import numpy as np
import ml_dtypes

BF16 = ml_dtypes.bfloat16

N = 100000
NPG = 1000
B_GRAPHS = 100
H = 128
CIN = 3
OUT = 2
NCORES = 8
SH = 12500          # real nodes per core
NB = 4              # src blocks
BLK = 25000         # nodes per conv1 src block (xtab, node-major)
NG = 98             # dst groups of 128 per core (last group = 84 real)
SHPAD = NG * 128    # 12544
RB = 2 * SHPAD      # rows per conv2 src block (2 ranks, padded) = 25088
NTAB = NCORES * SHPAD
GS = 4              # dst groups per super-group
T = 100
BL = 125            # batch lanes per core
GMAX = int(os.environ.get("K_GMAX", "896"))     # max idx per dma_gather
SCRATCH = int(os.environ.get("K_SCRATCH", "16384"))  # SWDGE ring bytes/part

_BUILT = {}


# --------------------------------------------------------------------------
# host preprocessing
# --------------------------------------------------------------------------
def _perm():
    n = np.arange(N)
    c = (n % NPG) // BL
    return c * SH + (n // NPG) * BL + (n % NPG) % BL


def _host_prep(inputs):
    x = np.asarray(inputs["x"], np.float32)
    ei = np.asarray(inputs["edge_index"])
    src, dst = ei[0].astype(np.int64), ei[1].astype(np.int64)
    p = _perm()
    srcp = p[src]
    dstp = p[dst]

    deg = np.bincount(dstp, minlength=N).astype(np.float32)
    recip = 1.0 / np.maximum(deg, 1.0)

    # conv2 table row for perm-id n: rank c, local L -> c*SHPAD + (L%128)*NG
    # + L//128 (partition-major so phase-3 writes are contiguous per lane)
    ids = np.arange(N)
    rowmap = (ids // SH) * SHPAD + (ids % SH % 128) * NG + (ids % SH // 128)

    owner = dstp // SH
    W = np.zeros((NG, NB), np.int64)
    per_core = []
    for c in range(NCORES):
        m = owner == c
        L = dstp[m] - c * SH
        g = L // 128
        slot = (L % 128).astype(np.float32)
        sp = srcp[m]
        b = sp // BLK
        s1 = (sp % BLK).astype(np.int16)
        s2 = (rowmap[sp] % RB).astype(np.int16)
        key = (g * NB + b).astype(np.int64)
        order = np.argsort(key, kind="stable")
        cnt = np.bincount(key, minlength=NG * NB)
        per_core.append((s1[order], s2[order], slot[order], key[order], cnt))
        W = np.maximum(W, cnt.reshape(NG, NB))
    # round bucket widths to 64 so every chunk segment starts at partition
    # 0 or 64 (PE tile-position constraint)
    W = ((np.maximum(W, 1) + 63) // 64) * 64

    # tight slot layout: for sup: for b: groups packed back-to-back at their
    # exact max-over-cores widths; each run padded to a chunk (128) multiple.
    # Chunks may straddle group boundaries -> per-chunk segment lists.
    sups = [range(i, min(i + GS, NG)) for i in range(0, NG, GS)]
    sbase = np.zeros((NG, NB), np.int64)
    gmeta = []
    nch = 0
    for sup in sups:
        sup_base = nch
        bruns = []
        supsegs = []
        for b in range(NB):
            run0 = nch
            off = 0
            offs = []
            for g in sup:
                sbase[g, b] = run0 * 128 + off
                offs.append((g, off, off + int(W[g, b])))
                off += int(W[g, b])
            nch_b = (off + 127) // 128
            nch += nch_b
            bruns.append((b, run0, nch_b))
            segs = []
            for gi, (g, o0, o1) in enumerate(offs):
                k0, k1 = o0 // 128, (o1 - 1) // 128
                for k in range(k0, k1 + 1):
                    r0 = max(o0, k * 128) - k * 128
                    r1 = min(o1, (k + 1) * 128) - k * 128
                    segs.append((k, gi, r0, r1))
            supsegs.append(segs)
        # regroup segments per dst group: each group's accumulation chain
        # must be emitted contiguously (one open PSUM group at a time)
        byg = [[] for _ in sup]
        for bi, segs in enumerate(supsegs):
            for (k, gi, r0, r1) in segs:
                byg[gi].append((bi, k, r0, r1))
        gmeta.append((sup_base, nch - sup_base, bruns, byg))
    NCH = nch
    NSL = NCH * 128
    NBMAX = max(nb for (_, _, brs, _) in gmeta for (_, _, nb) in brs)

    percore = []
    base_of_key = sbase.reshape(-1)
    for c in range(NCORES):
        s1o, s2o, slot_o, key_o, cnt = per_core[c]
        run_start = np.concatenate([[0], np.cumsum(cnt)[:-1]])
        rank_within = np.arange(len(s1o)) - run_start[key_o]
        pos = base_of_key[key_o] + rank_within
        idx1 = np.zeros(NSL, np.int16)
        idx2 = np.zeros(NSL, np.int16)
        dm_flat = np.full(NSL, -1.0, np.float32)
        idx1[pos] = s1o
        idx2[pos] = s2o
        dm_flat[pos] = slot_o

        def wrap(v):
            w = v.reshape(NSL // 16, 16).T
            return np.ascontiguousarray(np.tile(w, (8, 1)).astype(np.int16))

        r = np.ones(SHPAD, np.float32)
        r[:SH] = recip[c * SH:(c + 1) * SH]
        percore.append({
            "idx16a": wrap(idx1),
            "idx16b": wrap(idx2),
            "dmv": np.ascontiguousarray(dm_flat.reshape(NCH, 128).T
                                        .astype(BF16)),
            "recbT": np.ascontiguousarray(
                np.broadcast_to(r, (128, SHPAD)).astype(BF16)),
        })

    # tables / weights in perm order
    inv = np.empty(N, np.int64)
    inv[p] = np.arange(N)
    xp = np.zeros((N, H), np.float32)
    xp[:, :CIN] = x[inv]
    xp[:, CIN] = 1.0
    x4T = np.zeros((4, N + 96), np.float32)
    x4T[:, :N] = xp[:, :4].T
    for c in range(NCORES):
        xl = np.zeros((4, SHPAD), np.float32)
        xl[:, :SH] = x4T[:, c * SH:(c + 1) * SH]
        percore[c]["x4tloc"] = xl.astype(BF16)

    Wcomb = np.zeros((8, H), np.float32)
    Wcomb[0:3] = np.asarray(inputs["W_self0"], np.float32)
    Wcomb[3] = np.asarray(inputs["b0"], np.float32)
    Wcomb[4:7] = np.asarray(inputs["W_nbr0"], np.float32)

    shared = {
        "xtab": xp.astype(BF16),
        "x4T": x4T.astype(BF16),
        "wcomb": Wcomb.astype(BF16),
        "ws1": np.asarray(inputs["W_self1"], np.float32).astype(BF16),
        "wn1": np.asarray(inputs["W_nbr1"], np.float32).astype(BF16),
        "b1c": np.ascontiguousarray(
            np.asarray(inputs["b1"], np.float32).reshape(H, 1)),
        "wo": (np.asarray(inputs["W_out"], np.float32) / NPG).astype(BF16),
        "bo": np.ascontiguousarray(
            np.asarray(inputs["b_out"], np.float32).reshape(OUT, 1)),
    }
    # LSTM gate layout reordered to [i, f, o, g] so one sigmoid covers i,f,o
    GORD = [0, 1, 3, 2]
    bzs = []
    for l in range(2):
        wi = np.asarray(inputs[f"Wih{l}"], np.float32)
        wh = np.asarray(inputs[f"Whh{l}"], np.float32)
        bs = (np.asarray(inputs[f"bih{l}"], np.float32)
              + np.asarray(inputs[f"bhh{l}"], np.float32))
        shared[f"wi{l}"] = np.ascontiguousarray(np.concatenate(
            [wi[q * H:(q + 1) * H].T for q in GORD], axis=1)).astype(BF16)
        shared[f"wh{l}"] = np.ascontiguousarray(np.concatenate(
            [wh[q * H:(q + 1) * H].T for q in GORD], axis=1)).astype(BF16)
        bsbc = np.zeros((H, 4 * BL), np.float32)
        for qi, q in enumerate(GORD):
            bsbc[:, qi * BL:(qi + 1) * BL] = bs[q * H:(q + 1) * H][:, None]
        shared[f"bs{l}"] = bsbc.astype(BF16)
        bzs.append(bool(np.all(bs == 0.0)))

    meta = tuple(W.reshape(-1).tolist()) + tuple(bzs)
    return shared, percore, meta, gmeta, NCH, NBMAX, bzs


# --------------------------------------------------------------------------
# device program
# --------------------------------------------------------------------------
def _build_nc(gmeta, NCH, NBMAX, bzs):
    import concourse.bacc as bacc
    import concourse.mybir as mybir
    from concourse.tile import TileContext
    from concourse.masks import make_identity

    f32 = mybir.dt.float32
    bf = mybir.dt.bfloat16
    i16 = mybir.dt.int16
    AF = mybir.ActivationFunctionType
    ALU = mybir.AluOpType
    NSL = NCH * 128
    sups = [range(i, min(i + GS, NG)) for i in range(0, NG, GS)]
    GW = NBMAX * 128     # gather/oh tile width (slots)

    nc = bacc.Bacc(None, target_bir_lowering=False,
                   dynamic_dma_scratch_size=SCRATCH)

    d_xtab = nc.dram_tensor("xtab", [N, H], bf, kind="ExternalInput")
    d_x4T = nc.dram_tensor("x4T", [4, N + 96], bf, kind="ExternalInput")
    d_wcomb = nc.dram_tensor("wcomb", [8, H], bf, kind="ExternalInput")
    d_ws1 = nc.dram_tensor("ws1", [H, H], bf, kind="ExternalInput")
    d_wn1 = nc.dram_tensor("wn1", [H, H], bf, kind="ExternalInput")
    d_b1c = nc.dram_tensor("b1c", [H, 1], f32, kind="ExternalInput")
    d_wo = nc.dram_tensor("wo", [H, OUT], bf, kind="ExternalInput")
    d_bo = nc.dram_tensor("bo", [OUT, 1], f32, kind="ExternalInput")
    d_wi = [nc.dram_tensor(f"wi{l}", [H, 4 * H], bf, kind="ExternalInput")
            for l in range(2)]
    d_wh = [nc.dram_tensor(f"wh{l}", [H, 4 * H], bf, kind="ExternalInput")
            for l in range(2)]
    d_bs = [nc.dram_tensor(f"bs{l}", [H, 4 * BL], bf, kind="ExternalInput")
            for l in range(2)]
    d_idxa = nc.dram_tensor("idx16a", [128, NSL // 16], i16,
                            kind="ExternalInput")
    d_idxb = nc.dram_tensor("idx16b", [128, NSL // 16], i16,
                            kind="ExternalInput")
    d_dmv = nc.dram_tensor("dmv", [128, NCH], bf, kind="ExternalInput")
    d_recbT = nc.dram_tensor("recbT", [128, SHPAD], bf, kind="ExternalInput")
    d_x4tloc = nc.dram_tensor("x4tloc", [4, SHPAD], bf, kind="ExternalInput")
    d_out = nc.dram_tensor("out", [B_GRAPHS, OUT], f32, kind="ExternalOutput")

    with TileContext(nc) as tc:
        with (
            tc.tile_pool(name="dram", bufs=1, space="DRAM") as dramp,
            tc.tile_pool(name="persist", bufs=1) as pers,
        ):
            h1tab = dramp.tile([NTAB, H], bf)
            cc_in = dramp.tile([4, SHPAD], bf)
            cc_out = dramp.tile([4 * NCORES, SHPAD], bf, addr_space="Shared")
            ccr_in = dramp.tile([OUT, B_GRAPHS], f32)
            ccr_out = dramp.tile([OUT, B_GRAPHS], f32, addr_space="Shared")

            h2T = pers.tile([H, SHPAD], bf)
            h1Tl = pers.tile([H, SHPAD], bf)
            aggnT = pers.tile([4, SHPAD], bf)
            recbT = pers.tile([128, SHPAD], bf)
            w_comb = pers.tile([8, H], bf)
            w_s1 = pers.tile([H, H], bf)
            w_n1 = pers.tile([H, H], bf)
            b1c = pers.tile([H, 1], f32)
            w_i = [pers.tile([H, 4 * H], bf, name=f"w_i{l}") for l in range(2)]
            w_h = [pers.tile([H, 4 * H], bf, name=f"w_h{l}") for l in range(2)]
            b_s = [pers.tile([H, 4 * BL], bf, name=f"b_s{l}")
                   for l in range(2)]
            identb = pers.tile([128, 128], bf)
            w_o = pers.tile([H, OUT], bf)
            b_o = pers.tile([OUT, 1], f32)
            iotar = pers.tile([128, GW], bf)
            identf = pers.tile([OUT, OUT], f32)
            pooledT = pers.tile([H, B_GRAPHS], f32)

            nc.sync.dma_start(out=w_comb[:], in_=d_wcomb[:])
            nc.sync.dma_start(out=w_s1[:], in_=d_ws1[:])
            nc.sync.dma_start(out=w_n1[:], in_=d_wn1[:])
            nc.sync.dma_start(out=b1c[:], in_=d_b1c[:])
            for l in range(2):
                nc.sync.dma_start(out=w_i[l][:], in_=d_wi[l][:])
                nc.sync.dma_start(out=w_h[l][:], in_=d_wh[l][:])
                nc.sync.dma_start(out=b_s[l][:], in_=d_bs[l][:])
            nc.sync.dma_start(out=w_o[:], in_=d_wo[:])
            nc.sync.dma_start(out=b_o[:], in_=d_bo[:])
            nc.sync.dma_start(out=recbT[:], in_=d_recbT[:])
            make_identity(nc, identf[:])
            make_identity(nc, identb[:])
            with tc.tile_pool(name="tmpiota", bufs=1) as tmpp:
                io32 = tmpp.tile([128, GW], mybir.dt.int32)
                nc.gpsimd.iota(
                    io32[:].rearrange("p (j k) -> p j k", k=NBMAX),
                    pattern=[[1, 128], [0, NBMAX]], base=0,
                    channel_multiplier=0)
                nc.vector.tensor_copy(out=iotar[:], in_=io32[:])

            # -------------- generic conv phase ---------------------------
            def conv_phase(table_of, d_idx, gpool, ohpool, mpool,
                           emit_mm, sup_post):
                for si, (sup0, nch_sup, bruns, byg) in enumerate(gmeta):
                    sup = sups[si]
                    dm_t = mpool.tile([128, GS * NBMAX], bf, tag="dm")
                    nc.sync.dma_start(
                        out=dm_t[:, :nch_sup],
                        in_=d_dmv[:, sup0:sup0 + nch_sup])
                    gts = {}
                    for (b, run0, nch_b) in bruns:
                        n_idx = nch_b * 128
                        it = mpool.tile([128, GW // 16], i16, tag=f"ix{b}")
                        nc.sync.dma_start(
                            out=it[:, :n_idx // 16],
                            in_=d_idx[:, run0 * 8:run0 * 8 + n_idx // 16])
                        gt = gpool.tile([128, GW], bf, tag="g")
                        for o in range(0, n_idx, GMAX):
                            nn_ = min(GMAX, n_idx - o)
                            nc.gpsimd.dma_gather(
                                out_ap=gt[:, o:o + nn_]
                                    .rearrange("p (k h) -> p k h", h=H),
                                in_ap=table_of(b),
                                idxs_ap=it[:, o // 16:(o + nn_) // 16],
                                num_idxs=nn_,
                                num_idxs_reg=nn_,
                                elem_size=H,
                            )
                        gts[b] = gt
                    oh3s = []
                    for bi, (b, run0, nch_b) in enumerate(bruns):
                        oh = ohpool.tile([128, GW], bf, tag="oh")
                        nc.vector.tensor_tensor(
                            out=oh[:, :128 * nch_b]
                                .rearrange("p (j k) -> p j k", k=nch_b),
                            in0=dm_t[:, run0 - sup0:run0 - sup0 + nch_b]
                                .unsqueeze(1)
                                .broadcast_to([128, 128, nch_b]),
                            in1=iotar[:].rearrange("p (j k) -> p j k",
                                                   k=NBMAX)[:, :, 0:nch_b],
                            op=ALU.is_equal)
                        oh3s.append(oh[:, :128 * nch_b].rearrange(
                            "p (j k) -> p j k", k=nch_b))
                    for gi, g in enumerate(sup):
                        segs = byg[gi]
                        for ix, (bi, kin, r0, r1) in enumerate(segs):
                            b = bruns[bi][0]
                            emit_mm(si, gi, g,
                                    gts[b][r0:r1, kin * H:(kin + 1) * H],
                                    oh3s[bi][r0:r1, :, kin],
                                    ix == 0, ix == len(segs) - 1)
                    sup_post(si, sup)

            # ---------------- Phase 1: conv1 aggregation -----------------
            with (
                tc.tile_pool(name="p1g", bufs=5) as gpool,
                tc.tile_pool(name="p1oh", bufs=5) as ohpool,
                tc.tile_pool(name="p1m", bufs=3) as mpool,
                tc.tile_pool(name="p1ps", bufs=2, space="PSUM") as pspool,
            ):
                cur = {}

                def mm1(si, gi, g, g_ap, oh_ap, first, last):
                    if gi == 0 and first:
                        cur["ps"] = pspool.tile([4, GS * 128], f32,
                                                space="PSUM", tag="agg1",
                                                name="agg1ps")
                    nc.tensor.matmul(
                        out=cur["ps"][:, gi * 128:(gi + 1) * 128],
                        lhsT=g_ap[:, 0:4], rhs=oh_ap,
                        start=first, stop=last)

                def post1(si, sup):
                    w = len(sup) * 128
                    c0 = sup[0] * 128
                    nc.vector.tensor_tensor(
                        out=aggnT[:, c0:c0 + w], in0=cur["ps"][:, :w],
                        in1=recbT[0:4, c0:c0 + w], op=ALU.mult)

                conv_phase(lambda b: d_xtab[b * BLK:(b + 1) * BLK, :],
                           d_idxa, gpool, ohpool, mpool, mm1, post1)

            nc.sync.dma_start(out=cc_in[:], in_=aggnT[:])
            nc.gpsimd.collective_compute(
                "AllGather", mybir.AluOpType.bypass,
                replica_groups=[list(range(NCORES))],
                ins=[cc_in.opt()], outs=[cc_out.opt()],
            )

            # -------- Phase 3: recompute h1 (all ranks) + local h1T ------
            QJ = 24    # j-groups per phase-3 write chunk
            with (
                tc.tile_pool(name="p3xal", bufs=1) as xalpool,
                tc.tile_pool(name="p3xa", bufs=4) as xapool,
                tc.tile_pool(name="p3h", bufs=4) as hpool,
                tc.tile_pool(name="p3psw", bufs=1, space="PSUM") as pswp,
                tc.tile_pool(name="p3psj", bufs=6, space="PSUM") as psjp,
            ):
                xal = xalpool.tile([8, SHPAD], bf)
                nc.sync.dma_start(out=xal[0:4, :], in_=d_x4tloc[:])
                nc.sync.dma_start(out=xal[4:8, :], in_=aggnT[:])
                for j0 in range(0, SHPAD, 512):
                    w = min(512, SHPAD - j0)
                    psw = pswp.tile([H, 512], f32, space="PSUM", tag="psw")
                    nc.tensor.matmul(out=psw[:, :w], lhsT=w_comb[:],
                                     rhs=xal[:, j0:j0 + w],
                                     start=True, stop=True)
                    nc.scalar.activation(h1Tl[:, j0:j0 + w], psw[:, :w],
                                         AF.Relu)

                eng = 0
                for r in range(NCORES):
                    for q0 in range(0, NG, QJ):
                        q1 = min(q0 + QJ, NG)
                        ncol = (q1 - q0) * 128
                        xa = xapool.tile([8, (QJ + 2) * 128], bf, tag="xa")
                        nc.sync.dma_start(
                            out=xa[0:4, :ncol],
                            in_=d_x4T[:, r * SH + q0 * 128:
                                      r * SH + q0 * 128 + ncol])
                        nc.sync.dma_start(
                            out=xa[4:8, :ncol],
                            in_=cc_out[4 * r:4 * r + 4,
                                       q0 * 128:q0 * 128 + ncol])
                        hb = hpool.tile([128, (QJ + 2) * 128], bf, tag="hb")
                        for jj0 in range(0, q1 - q0, 4):
                            nj = min(4, q1 - q0 - jj0)
                            ps = psjp.tile([128, 4 * H], f32, space="PSUM",
                                           tag="psj")
                            for jj in range(jj0, jj0 + nj):
                                nc.tensor.matmul(
                                    out=ps[:, (jj - jj0) * H:
                                           (jj - jj0 + 1) * H],
                                    lhsT=xa[:, jj * 128:(jj + 1) * 128],
                                    rhs=w_comb[:], start=True, stop=True)
                            hslc = hb[:, jj0 * 128:(jj0 + nj) * 128]
                            if eng == 0:
                                nc.scalar.activation(
                                    hslc, ps[:, :nj * H], AF.Relu)
                            else:
                                nc.vector.tensor_scalar(
                                    out=hslc, in0=ps[:, :nj * H], scalar1=0.0,
                                    scalar2=None, op0=ALU.max)
                            eng ^= 1
                        nc.sync.dma_start(
                            out=h1tab[r * SHPAD:(r + 1) * SHPAD, :]
                                .rearrange("(p j) h -> p j h", j=NG)
                                [:, q0:q1, :],
                            in_=hb[:, :ncol]
                                .rearrange("p (j h) -> p j h", h=H))

            # ---------------- Phase 4: conv2 + LSTM ----------------------
            with (
                tc.tile_pool(name="p4g", bufs=5) as gpool,
                tc.tile_pool(name="p4oh", bufs=5) as ohpool,
                tc.tile_pool(name="p4m", bufs=3) as mpool,
                tc.tile_pool(name="p4ps", bufs=2, space="PSUM") as pspool4,
                tc.tile_pool(name="p4ps2", bufs=2, space="PSUM") as pspool4b,
                tc.tile_pool(name="p4t", bufs=3) as tpool,
                tc.tile_pool(name="p5s", bufs=3) as spool,
                tc.tile_pool(name="p5ps", bufs=2, space="PSUM") as pslstm,
            ):
                cur = {}
                hprev = [None, None]
                cprev = [None, None]
                h1hist = {}

                def lstm_layer(t, l):
                    # gate layout [i, f, o, g]; biases pre-added via one
                    # identity matmul from the broadcast bias tile
                    xT = (h2T[:, t * BL:(t + 1) * BL] if l == 0
                          else h1hist[t][:])
                    gps = pslstm.tile([H, 4 * BL], f32, space="PSUM",
                                      tag=f"gl{l}", name="gps")
                    for q in range(4):
                        sl = gps[:, q * BL:(q + 1) * BL]
                        if not bzs[l]:
                            nc.tensor.matmul(
                                out=sl, lhsT=identb[:],
                                rhs=b_s[l][:, q * BL:(q + 1) * BL],
                                start=True, stop=False)
                        nc.tensor.matmul(
                            out=sl, lhsT=w_i[l][:, q * H:(q + 1) * H],
                            rhs=xT, start=bzs[l], stop=(t == 0))
                        if t > 0:
                            nc.tensor.matmul(
                                out=sl,
                                lhsT=w_h[l][:, q * H:(q + 1) * H],
                                rhs=hprev[l][:], start=False, stop=True)
                    sig3 = spool.tile([H, 3 * BL], bf, tag=f"s3{l}")
                    nc.scalar.activation(sig3[:], gps[:, 0:3 * BL],
                                         AF.Sigmoid)
                    tg = spool.tile([H, BL], bf, tag=f"tg{l}")
                    nc.scalar.activation(tg[:], gps[:, 3 * BL:4 * BL],
                                         AF.Tanh)
                    cnew = spool.tile([H, BL], bf, tag=f"c{l}")
                    if t > 0:
                        t1 = spool.tile([H, BL], bf, tag=f"t1{l}")
                        nc.vector.tensor_tensor(out=cnew[:],
                                                in0=sig3[:, BL:2 * BL],
                                                in1=cprev[l][:], op=ALU.mult)
                        nc.vector.tensor_tensor(out=t1[:],
                                                in0=sig3[:, 0:BL],
                                                in1=tg[:], op=ALU.mult)
                        nc.vector.tensor_tensor(out=cnew[:], in0=cnew[:],
                                                in1=t1[:], op=ALU.add)
                    else:
                        nc.vector.tensor_tensor(out=cnew[:],
                                                in0=sig3[:, 0:BL],
                                                in1=tg[:], op=ALU.mult)
                    tc_ = spool.tile([H, BL], bf, tag=f"tc{l}")
                    nc.scalar.activation(tc_[:], cnew[:], AF.Tanh)
                    hnew = spool.tile([H, BL], bf, tag=f"h{l}")
                    nc.vector.tensor_tensor(out=hnew[:],
                                            in0=sig3[:, 2 * BL:3 * BL],
                                            in1=tc_[:], op=ALU.mult)
                    cprev[l] = cnew
                    hprev[l] = hnew
                    if l == 0:
                        h1hist[t] = hnew
                        h1hist.pop(t - 3, None)
                    else:
                        nc.vector.tensor_reduce(
                            out=pooledT[:, t:t + 1], in_=hnew[:],
                            axis=mybir.AxisListType.X, op=ALU.add)

                def mm2(si, gi, g, g_ap, oh_ap, first, last):
                    if gi == 0 and first:
                        cur["ps"] = pspool4.tile([128, GS * 128], f32,
                                                 space="PSUM", tag="agg2",
                                                 name="agg2ps")
                    nc.tensor.matmul(
                        out=cur["ps"][:, gi * 128:(gi + 1) * 128],
                        lhsT=g_ap, rhs=oh_ap, start=first, stop=last)

                def post2(si, sup):
                    for gi, g in enumerate(sup):
                        aggTc = tpool.tile([H, 128], bf, tag="aggTc")
                        nc.vector.tensor_tensor(
                            out=aggTc[:],
                            in0=cur["ps"][:, gi * 128:(gi + 1) * 128],
                            in1=recbT[:, g * 128:(g + 1) * 128], op=ALU.mult)
                        ps2 = pspool4b.tile([H, 128], f32, space="PSUM",
                                            tag="h2")
                        nc.tensor.matmul(out=ps2[:], lhsT=w_s1[:],
                                         rhs=h1Tl[:, g * 128:(g + 1) * 128],
                                         start=True, stop=False)
                        nc.tensor.matmul(out=ps2[:], lhsT=w_n1[:],
                                         rhs=aggTc[:], start=False, stop=True)
                        nc.vector.tensor_scalar(
                            out=h2T[:, g * 128:(g + 1) * 128], in0=ps2[:],
                            scalar1=b1c[:, 0:1], scalar2=0.0,
                            op0=ALU.add, op1=ALU.max)
                        if g >= 1:
                            lstm_layer(g - 1, 1)
                        lstm_layer(g, 0)

                conv_phase(lambda b: h1tab[b * RB:(b + 1) * RB, :],
                           d_idxb, gpool, ohpool, mpool, mm2, post2)
                for t in range(NG, T):
                    lstm_layer(t - 1, 1)
                    lstm_layer(t, 0)
                lstm_layer(T - 1, 1)

            # ---------------- Phase 6: head ------------------------------
            with (
                tc.tile_pool(name="p6", bufs=1) as hp,
                tc.tile_pool(name="p6ps", bufs=1, space="PSUM") as psp,
            ):
                poolb = hp.tile([H, B_GRAPHS], bf)
                nc.vector.tensor_copy(out=poolb[:], in_=pooledT[:])
                psl = psp.tile([OUT, B_GRAPHS], f32, space="PSUM",
                               tag="lg")
                nc.tensor.matmul(out=psl[:], lhsT=w_o[:], rhs=poolb[:],
                                 start=True, stop=True)
                lgp = hp.tile([OUT, B_GRAPHS], f32)
                nc.vector.tensor_copy(out=lgp[:], in_=psl[:])
                nc.sync.dma_start(out=ccr_in[:], in_=lgp[:])
                nc.gpsimd.collective_compute(
                    "AllReduce", mybir.AluOpType.add,
                    replica_groups=[list(range(NCORES))],
                    ins=[ccr_in.opt()], outs=[ccr_out.opt()],
                )
                prT = hp.tile([OUT, B_GRAPHS], f32)
                nc.sync.dma_start(out=prT[:], in_=ccr_out[:])
                lg = hp.tile([OUT, B_GRAPHS], f32)
                nc.vector.tensor_scalar(out=lg[:], in0=prT[:],
                                        scalar1=b_o[:, 0:1], scalar2=None,
                                        op0=ALU.add)
                pst = psp.tile([B_GRAPHS, OUT], f32, space="PSUM",
                               tag="lgt")
                nc.tensor.transpose(out=pst[:], in_=lg[:],
                                    identity=identf[:])
                z = hp.tile([B_GRAPHS, OUT], f32)
                nc.vector.tensor_copy(out=z[:], in_=pst[:])
                m = hp.tile([B_GRAPHS, 1], f32)
                nc.vector.tensor_reduce(out=m[:], in_=z[:],
                                        axis=mybir.AxisListType.X,
                                        op=ALU.max)
                negm = hp.tile([B_GRAPHS, 1], f32)
                nc.vector.tensor_scalar(out=negm[:], in0=m[:],
                                        scalar1=-1.0, scalar2=None,
                                        op0=ALU.mult)
                e = hp.tile([B_GRAPHS, OUT], f32)
                se = hp.tile([B_GRAPHS, 1], f32)
                nc.scalar.activation(e[:], z[:], AF.Exp,
                                     bias=negm[:, 0:1], accum_out=se[:])
                ls = hp.tile([B_GRAPHS, 1], f32)
                nc.scalar.activation(ls[:], se[:], AF.Ln)
                o_sb = hp.tile([B_GRAPHS, OUT], f32)
                nc.vector.tensor_scalar(out=o_sb[:], in0=z[:],
                                        scalar1=m[:, 0:1],
                                        scalar2=ls[:, 0:1],
                                        op0=ALU.subtract,
                                        op1=ALU.subtract)
                nc.sync.dma_start(out=d_out[:], in_=o_sb[:])

    nc.compile()
    return nc


# --------------------------------------------------------------------------
# PJRT runner (built once, reused across calls)
# --------------------------------------------------------------------------
class _Runner:
    def __init__(self, nc, n_cores):
        import jax
        import concourse.mybir as mybir
        from jax.sharding import Mesh, PartitionSpec
        from jax.experimental.shard_map import shard_map
        from concourse.bass2jax import (
            _bass_exec_p, install_neuronx_cc_hook, partition_id_tensor)

        install_neuronx_cc_hook()
        self.n_cores = n_cores
        in_names, out_names, out_avals, zero_outs = [], [], [], []
        pname = nc.partition_id_tensor.name if nc.partition_id_tensor else None
        for alloc in nc.m.functions[0].allocations:
            if not isinstance(alloc, mybir.MemoryLocationSet):
                continue
            name = alloc.memorylocations[0].name
            if alloc.kind == "ExternalInput":
                if name != pname:
                    in_names.append(name)
            elif alloc.kind == "ExternalOutput":
                shape = tuple(alloc.tensor_shape)
                dtype = mybir.dt.np(alloc.dtype)
                out_names.append(name)
                out_avals.append(jax.core.ShapedArray(shape, dtype))
                zero_outs.append(np.zeros(shape, dtype))
        self.in_names, self.out_names = in_names, out_names
        self.out_avals, self.zero_outs = out_avals, zero_outs
        n_params, n_outs = len(in_names), len(out_names)
        all_in = list(in_names) + list(out_names) + ([pname] if pname else [])

        def _body(*args):
            operands = list(args)
            if pname is not None:
                operands.append(partition_id_tensor())
            return tuple(_bass_exec_p.bind(
                *operands, out_avals=tuple(out_avals),
                in_names=tuple(all_in), out_names=tuple(out_names),
                lowering_input_output_aliases=(),
                sim_require_finite=True, sim_require_nnan=True, nc=nc))

        devices = jax.devices()[:n_cores]
        mesh = Mesh(np.asarray(devices), ("core",))
        self._jax = jax
        donate = (tuple(range(n_params, n_params + n_outs))
                  if devices[0].platform != "cpu" else ())
        self.sharded = jax.jit(
            shard_map(_body, mesh=mesh,
                      in_specs=(PartitionSpec("core"),) * (n_params + n_outs),
                      out_specs=(PartitionSpec("core"),) * n_outs,
                      check_rep=False),
            donate_argnums=donate,
            keep_unused=True)

    def concat_inputs(self, in_maps):
        return [np.concatenate([np.asarray(m[nm]) for m in in_maps], axis=0)
                for nm in self.in_names]

    def run(self, concat_in):
        zeros = [np.zeros((self.n_cores * z.shape[0], *z.shape[1:]), z.dtype)
                 for z in self.zero_outs]
        out = self.sharded(*concat_in, *zeros)
        self._jax.block_until_ready(out)
        return out

    def split(self, out_arrs):
        return [{nm: np.asarray(out_arrs[i]).reshape(
            self.n_cores, *self.out_avals[i].shape)[c]
            for i, nm in enumerate(self.out_names)}
            for c in range(self.n_cores)]


def kernel(**inputs):
    shared, percore, meta, gmeta, NCH, NBMAX, bzs = _host_prep(inputs)
    if meta not in _BUILT:
        nc = _build_nc(gmeta, NCH, NBMAX, bzs)
        _BUILT[meta] = (nc, _Runner(nc, NCORES))
    nc, runner = _BUILT[meta]
    in_maps = [dict(shared, **percore[c]) for c in range(NCORES)]
    ci = runner.concat_inputs(in_maps)
    outs = runner.split(runner.run(ci))
    return np.asarray(outs[0]["out"], np.float32)


# revision 51
# speedup vs baseline: 1.0564x; 1.0564x over previous
"""DCRNN kernel for 8 Trainium2 NeuronCores (Bass/Tile).

Graph/data-parallel sharding: nodes permuted so core c owns batch-lanes
[c*125,(c+1)*125) of every graph; edges partitioned by dst shard and bucketed
by (dst-group of 128, src-block) with cross-core-uniform chunk counts so one
SPMD program serves all 8 cores. Aggregation = dma_gather (one big gather per
(supergroup, src-block) run, int16 per-block indices) + one-hot matmul scatter
accumulating in PSUM. The one-hot is built in an interleaved [p, dst, chunk]
layout so every DVE operand is packed 2-byte SBUF (4x DVE mode). conv1's
4-wide transposed agg is AllGathered (tiny); every core then recomputes full
h1 (relu split across Act+DVE) and writes a partition-major bf16 h1 table
(12.5KB DMA descriptors) for conv2's gather. conv2 aggregates transposed
(lhsT=gathered rows) so the deg scaling is a free-dim multiply and h2 comes
out column-major for the LSTM. LSTM (bf16 weights/states) is interleaved with
conv2 group-by-group so it hides under conv2's DMA; global mean pool via
free-dim reduce, partial logits AllReduce (800B) + on-core log_softmax.
"""
import os
import numpy as np
import ml_dtypes

BF16 = ml_dtypes.bfloat16

N = 100000
NPG = 1000
B_GRAPHS = 100
H = 128
CIN = 3
OUT = 2
NCORES = 8
SH = 12500          # real nodes per core
NB = 4              # src blocks
BLK = 25000         # nodes per conv1 src block (xtab, node-major)
NG = 98             # dst groups of 128 per core (last group = 84 real)
SHPAD = NG * 128    # 12544
RB = 2 * SHPAD      # rows per conv2 src block (2 ranks, padded) = 25088
NTAB = NCORES * SHPAD
GS = 4              # dst groups per super-group
T = 100
BL = 125            # batch lanes per core
GMAX = int(os.environ.get("K_GMAX", "896"))     # max idx per dma_gather
SCRATCH = int(os.environ.get("K_SCRATCH", "16384"))  # SWDGE ring bytes/part

_BUILT = {}


# --------------------------------------------------------------------------
# host preprocessing
# --------------------------------------------------------------------------
def _perm():
    n = np.arange(N)
    c = (n % NPG) // BL
    return c * SH + (n // NPG) * BL + (n % NPG) % BL


def _host_prep(inputs):
    x = np.asarray(inputs["x"], np.float32)
    ei = np.asarray(inputs["edge_index"])
    src, dst = ei[0].astype(np.int64), ei[1].astype(np.int64)
    p = _perm()
    srcp = p[src]
    dstp = p[dst]

    deg = np.bincount(dstp, minlength=N).astype(np.float32)
    recip = 1.0 / np.maximum(deg, 1.0)

    # conv2 table row for perm-id n: rank c, local L -> c*SHPAD + (L%128)*NG
    # + L//128 (partition-major so phase-3 writes are contiguous per lane)
    ids = np.arange(N)
    rowmap = (ids // SH) * SHPAD + (ids % SH % 128) * NG + (ids % SH // 128)

    owner = dstp // SH
    W = np.zeros((NG, NB), np.int64)
    per_core = []
    for c in range(NCORES):
        m = owner == c
        L = dstp[m] - c * SH
        g = L // 128
        slot = (L % 128).astype(np.float32)
        sp = srcp[m]
        b = sp // BLK
        s1 = (sp % BLK).astype(np.int16)
        s2 = (rowmap[sp] % RB).astype(np.int16)
        key = (g * NB + b).astype(np.int64)
        order = np.argsort(key, kind="stable")
        cnt = np.bincount(key, minlength=NG * NB)
        per_core.append((s1[order], s2[order], slot[order], key[order], cnt))
        W = np.maximum(W, cnt.reshape(NG, NB))
    # round bucket widths to 64 so every chunk segment starts at partition
    # 0 or 64 (PE tile-position constraint)
    W = ((np.maximum(W, 1) + 63) // 64) * 64

    # tight slot layout: for sup: for b: groups packed back-to-back at their
    # exact max-over-cores widths; each run padded to a chunk (128) multiple.
    # Chunks may straddle group boundaries -> per-chunk segment lists.
    sups = [range(i, min(i + GS, NG)) for i in range(0, NG, GS)]
    sbase = np.zeros((NG, NB), np.int64)
    gmeta = []
    nch = 0
    for sup in sups:
        sup_base = nch
        bruns = []
        supsegs = []
        for b in range(NB):
            run0 = nch
            off = 0
            offs = []
            for g in sup:
                sbase[g, b] = run0 * 128 + off
                offs.append((g, off, off + int(W[g, b])))
                off += int(W[g, b])
            nch_b = (off + 127) // 128
            nch += nch_b
            bruns.append((b, run0, nch_b))
            segs = []
            for gi, (g, o0, o1) in enumerate(offs):
                k0, k1 = o0 // 128, (o1 - 1) // 128
                for k in range(k0, k1 + 1):
                    r0 = max(o0, k * 128) - k * 128
                    r1 = min(o1, (k + 1) * 128) - k * 128
                    segs.append((k, gi, r0, r1))
            supsegs.append(segs)
        # regroup segments per dst group: each group's accumulation chain
        # must be emitted contiguously (one open PSUM group at a time)
        byg = [[] for _ in sup]
        for bi, segs in enumerate(supsegs):
            for (k, gi, r0, r1) in segs:
                byg[gi].append((bi, k, r0, r1))
        gmeta.append((sup_base, nch - sup_base, bruns, byg))
    NCH = nch
    NSL = NCH * 128
    NBMAX = max(nb for (_, _, brs, _) in gmeta for (_, _, nb) in brs)

    percore = []
    base_of_key = sbase.reshape(-1)
    for c in range(NCORES):
        s1o, s2o, slot_o, key_o, cnt = per_core[c]
        run_start = np.concatenate([[0], np.cumsum(cnt)[:-1]])
        rank_within = np.arange(len(s1o)) - run_start[key_o]
        pos = base_of_key[key_o] + rank_within
        idx1 = np.zeros(NSL, np.int16)
        idx2 = np.zeros(NSL, np.int16)
        dm_flat = np.full(NSL, -1.0, np.float32)
        idx1[pos] = s1o
        idx2[pos] = s2o
        dm_flat[pos] = slot_o

        def wrap(v):
            w = v.reshape(NSL // 16, 16).T
            return np.ascontiguousarray(np.tile(w, (8, 1)).astype(np.int16))

        r = np.ones(SHPAD, np.float32)
        r[:SH] = recip[c * SH:(c + 1) * SH]
        percore.append({
            "idx16a": wrap(idx1),
            "idx16b": wrap(idx2),
            "dmv": np.ascontiguousarray(dm_flat.reshape(NCH, 128).T
                                        .astype(BF16)),
            "recbT": np.ascontiguousarray(
                np.broadcast_to(r, (128, SHPAD)).astype(BF16)),
        })

    # tables / weights in perm order
    inv = np.empty(N, np.int64)
    inv[p] = np.arange(N)
    xp = np.zeros((N, H), np.float32)
    xp[:, :CIN] = x[inv]
    xp[:, CIN] = 1.0
    x4T = np.zeros((4, N + 96), np.float32)
    x4T[:, :N] = xp[:, :4].T
    for c in range(NCORES):
        xl = np.zeros((4, SHPAD), np.float32)
        xl[:, :SH] = x4T[:, c * SH:(c + 1) * SH]
        percore[c]["x4tloc"] = xl.astype(BF16)

    Wcomb = np.zeros((8, H), np.float32)
    Wcomb[0:3] = np.asarray(inputs["W_self0"], np.float32)
    Wcomb[3] = np.asarray(inputs["b0"], np.float32)
    Wcomb[4:7] = np.asarray(inputs["W_nbr0"], np.float32)

    shared = {
        "xtab": xp.astype(BF16),
        "x4T": x4T.astype(BF16),
        "wcomb": Wcomb.astype(BF16),
        "ws1": np.asarray(inputs["W_self1"], np.float32).astype(BF16),
        "wn1": np.asarray(inputs["W_nbr1"], np.float32).astype(BF16),
        "b1c": np.ascontiguousarray(
            np.asarray(inputs["b1"], np.float32).reshape(H, 1)),
        "wo": (np.asarray(inputs["W_out"], np.float32) / NPG).astype(BF16),
        "bo": np.ascontiguousarray(
            np.asarray(inputs["b_out"], np.float32).reshape(OUT, 1)),
    }
    # LSTM gate layout reordered to [i, f, o, g] so one sigmoid covers i,f,o
    GORD = [0, 1, 3, 2]
    bzs = []
    for l in range(2):
        wi = np.asarray(inputs[f"Wih{l}"], np.float32)
        wh = np.asarray(inputs[f"Whh{l}"], np.float32)
        bs = (np.asarray(inputs[f"bih{l}"], np.float32)
              + np.asarray(inputs[f"bhh{l}"], np.float32))
        shared[f"wi{l}"] = np.ascontiguousarray(np.concatenate(
            [wi[q * H:(q + 1) * H].T for q in GORD], axis=1)).astype(BF16)
        shared[f"wh{l}"] = np.ascontiguousarray(np.concatenate(
            [wh[q * H:(q + 1) * H].T for q in GORD], axis=1)).astype(BF16)
        bsbc = np.zeros((H, 4 * BL), np.float32)
        for qi, q in enumerate(GORD):
            bsbc[:, qi * BL:(qi + 1) * BL] = bs[q * H:(q + 1) * H][:, None]
        shared[f"bs{l}"] = bsbc.astype(BF16)
        bzs.append(bool(np.all(bs == 0.0)))

    meta = tuple(W.reshape(-1).tolist()) + tuple(bzs)
    return shared, percore, meta, gmeta, NCH, NBMAX, bzs


# --------------------------------------------------------------------------
# device program
# --------------------------------------------------------------------------
def _build_nc(gmeta, NCH, NBMAX, bzs):
    import concourse.bacc as bacc
    import concourse.mybir as mybir
    from concourse.tile import TileContext
    from concourse.masks import make_identity

    f32 = mybir.dt.float32
    bf = mybir.dt.bfloat16
    i16 = mybir.dt.int16
    AF = mybir.ActivationFunctionType
    ALU = mybir.AluOpType
    NSL = NCH * 128
    sups = [range(i, min(i + GS, NG)) for i in range(0, NG, GS)]
    GW = NBMAX * 128     # gather/oh tile width (slots)

    nc = bacc.Bacc(None, target_bir_lowering=False,
                   dynamic_dma_scratch_size=SCRATCH)

    d_xtab = nc.dram_tensor("xtab", [N, H], bf, kind="ExternalInput")
    d_x4T = nc.dram_tensor("x4T", [4, N + 96], bf, kind="ExternalInput")
    d_wcomb = nc.dram_tensor("wcomb", [8, H], bf, kind="ExternalInput")
    d_ws1 = nc.dram_tensor("ws1", [H, H], bf, kind="ExternalInput")
    d_wn1 = nc.dram_tensor("wn1", [H, H], bf, kind="ExternalInput")
    d_b1c = nc.dram_tensor("b1c", [H, 1], f32, kind="ExternalInput")
    d_wo = nc.dram_tensor("wo", [H, OUT], bf, kind="ExternalInput")
    d_bo = nc.dram_tensor("bo", [OUT, 1], f32, kind="ExternalInput")
    d_wi = [nc.dram_tensor(f"wi{l}", [H, 4 * H], bf, kind="ExternalInput")
            for l in range(2)]
    d_wh = [nc.dram_tensor(f"wh{l}", [H, 4 * H], bf, kind="ExternalInput")
            for l in range(2)]
    d_bs = [nc.dram_tensor(f"bs{l}", [H, 4 * BL], bf, kind="ExternalInput")
            for l in range(2)]
    d_idxa = nc.dram_tensor("idx16a", [128, NSL // 16], i16,
                            kind="ExternalInput")
    d_idxb = nc.dram_tensor("idx16b", [128, NSL // 16], i16,
                            kind="ExternalInput")
    d_dmv = nc.dram_tensor("dmv", [128, NCH], bf, kind="ExternalInput")
    d_recbT = nc.dram_tensor("recbT", [128, SHPAD], bf, kind="ExternalInput")
    d_x4tloc = nc.dram_tensor("x4tloc", [4, SHPAD], bf, kind="ExternalInput")
    d_out = nc.dram_tensor("out", [B_GRAPHS, OUT], f32, kind="ExternalOutput")

    with TileContext(nc) as tc:
        with (
            tc.tile_pool(name="dram", bufs=1, space="DRAM") as dramp,
            tc.tile_pool(name="persist", bufs=1) as pers,
        ):
            h1tab = dramp.tile([NTAB, H], bf)
            cc_in = dramp.tile([4, SHPAD], bf)
            cc_out = dramp.tile([4 * NCORES, SHPAD], bf, addr_space="Shared")
            ccr_in = dramp.tile([OUT, B_GRAPHS], f32)
            ccr_out = dramp.tile([OUT, B_GRAPHS], f32, addr_space="Shared")

            h2T = pers.tile([H, SHPAD], bf)
            h1Tl = pers.tile([H, SHPAD], bf)
            aggnT = pers.tile([4, SHPAD], bf)
            recbT = pers.tile([128, SHPAD], bf)
            w_comb = pers.tile([8, H], bf)
            w_s1 = pers.tile([H, H], bf)
            w_n1 = pers.tile([H, H], bf)
            b1c = pers.tile([H, 1], f32)
            w_i = [pers.tile([H, 4 * H], bf, name=f"w_i{l}") for l in range(2)]
            w_h = [pers.tile([H, 4 * H], bf, name=f"w_h{l}") for l in range(2)]
            b_s = [pers.tile([H, 4 * BL], bf, name=f"b_s{l}")
                   for l in range(2)]
            identb = pers.tile([128, 128], bf)
            w_o = pers.tile([H, OUT], bf)
            b_o = pers.tile([OUT, 1], f32)
            iotar = pers.tile([128, GW], bf)
            identf = pers.tile([OUT, OUT], f32)
            pooledT = pers.tile([H, B_GRAPHS], f32)

            nc.sync.dma_start(out=w_comb[:], in_=d_wcomb[:])
            nc.sync.dma_start(out=w_s1[:], in_=d_ws1[:])
            nc.sync.dma_start(out=w_n1[:], in_=d_wn1[:])
            nc.sync.dma_start(out=b1c[:], in_=d_b1c[:])
            for l in range(2):
                nc.sync.dma_start(out=w_i[l][:], in_=d_wi[l][:])
                nc.sync.dma_start(out=w_h[l][:], in_=d_wh[l][:])
                nc.sync.dma_start(out=b_s[l][:], in_=d_bs[l][:])
            nc.sync.dma_start(out=w_o[:], in_=d_wo[:])
            nc.sync.dma_start(out=b_o[:], in_=d_bo[:])
            nc.sync.dma_start(out=recbT[:], in_=d_recbT[:])
            make_identity(nc, identf[:])
            make_identity(nc, identb[:])
            with tc.tile_pool(name="tmpiota", bufs=1) as tmpp:
                io32 = tmpp.tile([128, GW], mybir.dt.int32)
                nc.gpsimd.iota(
                    io32[:].rearrange("p (j k) -> p j k", k=NBMAX),
                    pattern=[[1, 128], [0, NBMAX]], base=0,
                    channel_multiplier=0)
                nc.vector.tensor_copy(out=iotar[:], in_=io32[:])

            # -------------- generic conv phase ---------------------------
            def conv_phase(table_of, d_idx, gpool, ohpool, mpool,
                           emit_mm, sup_post):
                for si, (sup0, nch_sup, bruns, byg) in enumerate(gmeta):
                    sup = sups[si]
                    dm_t = mpool.tile([128, GS * NBMAX], bf, tag="dm")
                    nc.sync.dma_start(
                        out=dm_t[:, :nch_sup],
                        in_=d_dmv[:, sup0:sup0 + nch_sup])
                    gts = {}
                    for (b, run0, nch_b) in bruns:
                        n_idx = nch_b * 128
                        it = mpool.tile([128, GW // 16], i16, tag=f"ix{b}")
                        nc.sync.dma_start(
                            out=it[:, :n_idx // 16],
                            in_=d_idx[:, run0 * 8:run0 * 8 + n_idx // 16])
                        gt = gpool.tile([128, GW], bf, tag="g")
                        for o in range(0, n_idx, GMAX):
                            nn_ = min(GMAX, n_idx - o)
                            nc.gpsimd.dma_gather(
                                out_ap=gt[:, o:o + nn_]
                                    .rearrange("p (k h) -> p k h", h=H),
                                in_ap=table_of(b),
                                idxs_ap=it[:, o // 16:(o + nn_) // 16],
                                num_idxs=nn_,
                                num_idxs_reg=nn_,
                                elem_size=H,
                            )
                        gts[b] = gt
                    oh3s = []
                    for bi, (b, run0, nch_b) in enumerate(bruns):
                        oh = ohpool.tile([128, GW], bf, tag="oh")
                        nc.vector.tensor_tensor(
                            out=oh[:, :128 * nch_b]
                                .rearrange("p (j k) -> p j k", k=nch_b),
                            in0=dm_t[:, run0 - sup0:run0 - sup0 + nch_b]
                                .unsqueeze(1)
                                .broadcast_to([128, 128, nch_b]),
                            in1=iotar[:].rearrange("p (j k) -> p j k",
                                                   k=NBMAX)[:, :, 0:nch_b],
                            op=ALU.is_equal)
                        oh3s.append(oh[:, :128 * nch_b].rearrange(
                            "p (j k) -> p j k", k=nch_b))
                    for gi, g in enumerate(sup):
                        segs = byg[gi]
                        for ix, (bi, kin, r0, r1) in enumerate(segs):
                            b = bruns[bi][0]
                            emit_mm(si, gi, g,
                                    gts[b][r0:r1, kin * H:(kin + 1) * H],
                                    oh3s[bi][r0:r1, :, kin],
                                    ix == 0, ix == len(segs) - 1)
                    sup_post(si, sup)

            # ---------------- Phase 1: conv1 aggregation -----------------
            with (
                tc.tile_pool(name="p1g", bufs=5) as gpool,
                tc.tile_pool(name="p1oh", bufs=5) as ohpool,
                tc.tile_pool(name="p1m", bufs=3) as mpool,
                tc.tile_pool(name="p1ps", bufs=2, space="PSUM") as pspool,
            ):
                cur = {}

                def mm1(si, gi, g, g_ap, oh_ap, first, last):
                    if gi == 0 and first:
                        cur["ps"] = pspool.tile([4, GS * 128], f32,
                                                space="PSUM", tag="agg1",
                                                name="agg1ps")
                    nc.tensor.matmul(
                        out=cur["ps"][:, gi * 128:(gi + 1) * 128],
                        lhsT=g_ap[:, 0:4], rhs=oh_ap,
                        start=first, stop=last)

                def post1(si, sup):
                    w = len(sup) * 128
                    c0 = sup[0] * 128
                    nc.vector.tensor_tensor(
                        out=aggnT[:, c0:c0 + w], in0=cur["ps"][:, :w],
                        in1=recbT[0:4, c0:c0 + w], op=ALU.mult)

                conv_phase(lambda b: d_xtab[b * BLK:(b + 1) * BLK, :],
                           d_idxa, gpool, ohpool, mpool, mm1, post1)

            nc.sync.dma_start(out=cc_in[:], in_=aggnT[:])
            nc.gpsimd.collective_compute(
                "AllGather", mybir.AluOpType.bypass,
                replica_groups=[list(range(NCORES))],
                ins=[cc_in.opt()], outs=[cc_out.opt()],
            )

            # -------- Phase 3: recompute h1 (all ranks) + local h1T ------
            QJ = 24    # j-groups per phase-3 write chunk
            with (
                tc.tile_pool(name="p3xal", bufs=1) as xalpool,
                tc.tile_pool(name="p3xa", bufs=4) as xapool,
                tc.tile_pool(name="p3h", bufs=4) as hpool,
                tc.tile_pool(name="p3psw", bufs=2, space="PSUM") as pswp,
                tc.tile_pool(name="p3psj", bufs=3, space="PSUM") as psjp,
            ):
                xal = xalpool.tile([8, SHPAD], bf)
                nc.sync.dma_start(out=xal[0:4, :], in_=d_x4tloc[:])
                nc.sync.dma_start(out=xal[4:8, :], in_=aggnT[:])
                for j0 in range(0, SHPAD, 512):
                    w = min(512, SHPAD - j0)
                    psw = pswp.tile([H, 512], f32, space="PSUM", tag="psw")
                    nc.tensor.matmul(out=psw[:, :w], lhsT=w_comb[:],
                                     rhs=xal[:, j0:j0 + w],
                                     start=True, stop=True)
                    nc.scalar.activation(h1Tl[:, j0:j0 + w], psw[:, :w],
                                         AF.Relu)

                eng = 0
                for r in range(NCORES):
                    for q0 in range(0, NG, QJ):
                        q1 = min(q0 + QJ, NG)
                        ncol = (q1 - q0) * 128
                        xa = xapool.tile([8, (QJ + 2) * 128], bf, tag="xa")
                        nc.sync.dma_start(
                            out=xa[0:4, :ncol],
                            in_=d_x4T[:, r * SH + q0 * 128:
                                      r * SH + q0 * 128 + ncol])
                        nc.sync.dma_start(
                            out=xa[4:8, :ncol],
                            in_=cc_out[4 * r:4 * r + 4,
                                       q0 * 128:q0 * 128 + ncol])
                        hb = hpool.tile([128, (QJ + 2) * 128], bf, tag="hb")
                        for jj0 in range(0, q1 - q0, 8):
                            nj = min(8, q1 - q0 - jj0)
                            ps = psjp.tile([128, 8 * H], f32, space="PSUM",
                                           tag="psj")
                            for jj in range(jj0, jj0 + nj):
                                nc.tensor.matmul(
                                    out=ps[:, (jj - jj0) * H:
                                           (jj - jj0 + 1) * H],
                                    lhsT=xa[:, jj * 128:(jj + 1) * 128],
                                    rhs=w_comb[:], start=True, stop=True)
                            hslc = hb[:, jj0 * 128:(jj0 + nj) * 128]
                            if eng == 0:
                                nc.scalar.activation(
                                    hslc, ps[:, :nj * H], AF.Relu)
                            else:
                                nc.vector.tensor_scalar(
                                    out=hslc, in0=ps[:, :nj * H], scalar1=0.0,
                                    scalar2=None, op0=ALU.max)
                            eng ^= 1
                        nc.sync.dma_start(
                            out=h1tab[r * SHPAD:(r + 1) * SHPAD, :]
                                .rearrange("(p j) h -> p j h", j=NG)
                                [:, q0:q1, :],
                            in_=hb[:, :ncol]
                                .rearrange("p (j h) -> p j h", h=H))

            # ---------------- Phase 4: conv2 + LSTM ----------------------
            with (
                tc.tile_pool(name="p4g", bufs=5) as gpool,
                tc.tile_pool(name="p4oh", bufs=5) as ohpool,
                tc.tile_pool(name="p4m", bufs=3) as mpool,
                tc.tile_pool(name="p4ps", bufs=2, space="PSUM") as pspool4,
                tc.tile_pool(name="p4ps2", bufs=2, space="PSUM") as pspool4b,
                tc.tile_pool(name="p4t", bufs=3) as tpool,
                tc.tile_pool(name="p5s", bufs=4) as spool,
                tc.tile_pool(name="p5ps", bufs=2, space="PSUM") as pslstm,
            ):
                cur = {}
                hprev = [None, None]
                cprev = [None, None]
                h1hist = {}

                def lstm_layer(t, l):
                    # gate layout [i, f, o, g]; biases pre-added via one
                    # identity matmul from the broadcast bias tile
                    xT = (h2T[:, t * BL:(t + 1) * BL] if l == 0
                          else h1hist[t][:])
                    gps = pslstm.tile([H, 4 * BL], f32, space="PSUM",
                                      tag=f"gl{l}", name="gps")
                    for q in range(4):
                        sl = gps[:, q * BL:(q + 1) * BL]
                        if not bzs[l]:
                            nc.tensor.matmul(
                                out=sl, lhsT=identb[:],
                                rhs=b_s[l][:, q * BL:(q + 1) * BL],
                                start=True, stop=False)
                        nc.tensor.matmul(
                            out=sl, lhsT=w_i[l][:, q * H:(q + 1) * H],
                            rhs=xT, start=bzs[l], stop=(t == 0))
                        if t > 0:
                            nc.tensor.matmul(
                                out=sl,
                                lhsT=w_h[l][:, q * H:(q + 1) * H],
                                rhs=hprev[l][:], start=False, stop=True)
                    sig3 = spool.tile([H, 3 * BL], bf, tag=f"s3{l}")
                    nc.scalar.activation(sig3[:], gps[:, 0:3 * BL],
                                         AF.Sigmoid)
                    tg = spool.tile([H, BL], bf, tag=f"tg{l}")
                    nc.scalar.activation(tg[:], gps[:, 3 * BL:4 * BL],
                                         AF.Tanh)
                    cnew = spool.tile([H, BL], bf, tag=f"c{l}")
                    if t > 0:
                        t1 = spool.tile([H, BL], bf, tag=f"t1{l}")
                        nc.vector.tensor_tensor(out=cnew[:],
                                                in0=sig3[:, BL:2 * BL],
                                                in1=cprev[l][:], op=ALU.mult)
                        nc.vector.tensor_tensor(out=t1[:],
                                                in0=sig3[:, 0:BL],
                                                in1=tg[:], op=ALU.mult)
                        nc.vector.tensor_tensor(out=cnew[:], in0=cnew[:],
                                                in1=t1[:], op=ALU.add)
                    else:
                        nc.vector.tensor_tensor(out=cnew[:],
                                                in0=sig3[:, 0:BL],
                                                in1=tg[:], op=ALU.mult)
                    tc_ = spool.tile([H, BL], bf, tag=f"tc{l}")
                    nc.scalar.activation(tc_[:], cnew[:], AF.Tanh)
                    hnew = spool.tile([H, BL], bf, tag=f"h{l}")
                    nc.vector.tensor_tensor(out=hnew[:],
                                            in0=sig3[:, 2 * BL:3 * BL],
                                            in1=tc_[:], op=ALU.mult)
                    cprev[l] = cnew
                    hprev[l] = hnew
                    if l == 0:
                        h1hist[t] = hnew
                        h1hist.pop(t - 3, None)
                    else:
                        nc.vector.tensor_reduce(
                            out=pooledT[:, t:t + 1], in_=hnew[:],
                            axis=mybir.AxisListType.X, op=ALU.add)

                def mm2(si, gi, g, g_ap, oh_ap, first, last):
                    if gi == 0 and first:
                        cur["ps"] = pspool4.tile([128, GS * 128], f32,
                                                 space="PSUM", tag="agg2",
                                                 name="agg2ps")
                    nc.tensor.matmul(
                        out=cur["ps"][:, gi * 128:(gi + 1) * 128],
                        lhsT=g_ap, rhs=oh_ap, start=first, stop=last)

                def post2(si, sup):
                    for gi, g in enumerate(sup):
                        aggTc = tpool.tile([H, 128], bf, tag="aggTc")
                        nc.vector.tensor_tensor(
                            out=aggTc[:],
                            in0=cur["ps"][:, gi * 128:(gi + 1) * 128],
                            in1=recbT[:, g * 128:(g + 1) * 128], op=ALU.mult)
                        ps2 = pspool4b.tile([H, 128], f32, space="PSUM",
                                            tag="h2")
                        nc.tensor.matmul(out=ps2[:], lhsT=w_s1[:],
                                         rhs=h1Tl[:, g * 128:(g + 1) * 128],
                                         start=True, stop=False)
                        nc.tensor.matmul(out=ps2[:], lhsT=w_n1[:],
                                         rhs=aggTc[:], start=False, stop=True)
                        nc.vector.tensor_scalar(
                            out=h2T[:, g * 128:(g + 1) * 128], in0=ps2[:],
                            scalar1=b1c[:, 0:1], scalar2=0.0,
                            op0=ALU.add, op1=ALU.max)
                        lstm_layer(g, 0)
                        if g >= 1:
                            lstm_layer(g - 1, 1)

                conv_phase(lambda b: h1tab[b * RB:(b + 1) * RB, :],
                           d_idxb, gpool, ohpool, mpool, mm2, post2)
                for t in range(NG, T):
                    lstm_layer(t, 0)
                    lstm_layer(t - 1, 1)
                lstm_layer(T - 1, 1)

            # ---------------- Phase 6: head ------------------------------
            with (
                tc.tile_pool(name="p6", bufs=1) as hp,
                tc.tile_pool(name="p6ps", bufs=1, space="PSUM") as psp,
            ):
                poolb = hp.tile([H, B_GRAPHS], bf)
                nc.vector.tensor_copy(out=poolb[:], in_=pooledT[:])
                psl = psp.tile([OUT, B_GRAPHS], f32, space="PSUM",
                               tag="lg")
                nc.tensor.matmul(out=psl[:], lhsT=w_o[:], rhs=poolb[:],
                                 start=True, stop=True)
                lgp = hp.tile([OUT, B_GRAPHS], f32)
                nc.vector.tensor_copy(out=lgp[:], in_=psl[:])
                nc.sync.dma_start(out=ccr_in[:], in_=lgp[:])
                nc.gpsimd.collective_compute(
                    "AllReduce", mybir.AluOpType.add,
                    replica_groups=[list(range(NCORES))],
                    ins=[ccr_in.opt()], outs=[ccr_out.opt()],
                )
                prT = hp.tile([OUT, B_GRAPHS], f32)
                nc.sync.dma_start(out=prT[:], in_=ccr_out[:])
                lg = hp.tile([OUT, B_GRAPHS], f32)
                nc.vector.tensor_scalar(out=lg[:], in0=prT[:],
                                        scalar1=b_o[:, 0:1], scalar2=None,
                                        op0=ALU.add)
                pst = psp.tile([B_GRAPHS, OUT], f32, space="PSUM",
                               tag="lgt")
                nc.tensor.transpose(out=pst[:], in_=lg[:],
                                    identity=identf[:])
                z = hp.tile([B_GRAPHS, OUT], f32)
                nc.vector.tensor_copy(out=z[:], in_=pst[:])
                m = hp.tile([B_GRAPHS, 1], f32)
                nc.vector.tensor_reduce(out=m[:], in_=z[:],
                                        axis=mybir.AxisListType.X,
                                        op=ALU.max)
                negm = hp.tile([B_GRAPHS, 1], f32)
                nc.vector.tensor_scalar(out=negm[:], in0=m[:],
                                        scalar1=-1.0, scalar2=None,
                                        op0=ALU.mult)
                e = hp.tile([B_GRAPHS, OUT], f32)
                se = hp.tile([B_GRAPHS, 1], f32)
                nc.scalar.activation(e[:], z[:], AF.Exp,
                                     bias=negm[:, 0:1], accum_out=se[:])
                ls = hp.tile([B_GRAPHS, 1], f32)
                nc.scalar.activation(ls[:], se[:], AF.Ln)
                o_sb = hp.tile([B_GRAPHS, OUT], f32)
                nc.vector.tensor_scalar(out=o_sb[:], in0=z[:],
                                        scalar1=m[:, 0:1],
                                        scalar2=ls[:, 0:1],
                                        op0=ALU.subtract,
                                        op1=ALU.subtract)
                nc.sync.dma_start(out=d_out[:], in_=o_sb[:])

    nc.compile()
    return nc


# --------------------------------------------------------------------------
# PJRT runner (built once, reused across calls)
# --------------------------------------------------------------------------
class _Runner:
    def __init__(self, nc, n_cores):
        import jax
        import concourse.mybir as mybir
        from jax.sharding import Mesh, PartitionSpec
        from jax.experimental.shard_map import shard_map
        from concourse.bass2jax import (
            _bass_exec_p, install_neuronx_cc_hook, partition_id_tensor)

        install_neuronx_cc_hook()
        self.n_cores = n_cores
        in_names, out_names, out_avals, zero_outs = [], [], [], []
        pname = nc.partition_id_tensor.name if nc.partition_id_tensor else None
        for alloc in nc.m.functions[0].allocations:
            if not isinstance(alloc, mybir.MemoryLocationSet):
                continue
            name = alloc.memorylocations[0].name
            if alloc.kind == "ExternalInput":
                if name != pname:
                    in_names.append(name)
            elif alloc.kind == "ExternalOutput":
                shape = tuple(alloc.tensor_shape)
                dtype = mybir.dt.np(alloc.dtype)
                out_names.append(name)
                out_avals.append(jax.core.ShapedArray(shape, dtype))
                zero_outs.append(np.zeros(shape, dtype))
        self.in_names, self.out_names = in_names, out_names
        self.out_avals, self.zero_outs = out_avals, zero_outs
        n_params, n_outs = len(in_names), len(out_names)
        all_in = list(in_names) + list(out_names) + ([pname] if pname else [])

        def _body(*args):
            operands = list(args)
            if pname is not None:
                operands.append(partition_id_tensor())
            return tuple(_bass_exec_p.bind(
                *operands, out_avals=tuple(out_avals),
                in_names=tuple(all_in), out_names=tuple(out_names),
                lowering_input_output_aliases=(),
                sim_require_finite=True, sim_require_nnan=True, nc=nc))

        devices = jax.devices()[:n_cores]
        mesh = Mesh(np.asarray(devices), ("core",))
        self._jax = jax
        donate = (tuple(range(n_params, n_params + n_outs))
                  if devices[0].platform != "cpu" else ())
        self.sharded = jax.jit(
            shard_map(_body, mesh=mesh,
                      in_specs=(PartitionSpec("core"),) * (n_params + n_outs),
                      out_specs=(PartitionSpec("core"),) * n_outs,
                      check_rep=False),
            donate_argnums=donate,
            keep_unused=True)

    def concat_inputs(self, in_maps):
        return [np.concatenate([np.asarray(m[nm]) for m in in_maps], axis=0)
                for nm in self.in_names]

    def run(self, concat_in):
        zeros = [np.zeros((self.n_cores * z.shape[0], *z.shape[1:]), z.dtype)
                 for z in self.zero_outs]
        out = self.sharded(*concat_in, *zeros)
        self._jax.block_until_ready(out)
        return out

    def split(self, out_arrs):
        return [{nm: np.asarray(out_arrs[i]).reshape(
            self.n_cores, *self.out_avals[i].shape)[c]
            for i, nm in enumerate(self.out_names)}
            for c in range(self.n_cores)]


def kernel(**inputs):
    shared, percore, meta, gmeta, NCH, NBMAX, bzs = _host_prep(inputs)
    if meta not in _BUILT:
        nc = _build_nc(gmeta, NCH, NBMAX, bzs)
        _BUILT[meta] = (nc, _Runner(nc, NCORES))
    nc, runner = _BUILT[meta]
    in_maps = [dict(shared, **percore[c]) for c in range(NCORES)]
    ci = runner.concat_inputs(in_maps)
    outs = runner.split(runner.run(ci))
    return np.asarray(outs[0]["out"], np.float32)
